# revision 16
# baseline (speedup 1.0000x reference)
"""Multi-head attention (B=4, S=1024, H=1024, 16 heads) on 8 trn2 cores.

Sharding: 8 shards = (batch b in 0..3) x (head-half hf in 0..1).
Each core computes attention for 8 heads of one batch and a partial
output projection (row-parallel Wo); host sums the two partials per batch.

v2 pipeline (per core), designed around three facts measured on v1:
  - ACT exp of 8.4M logits is the irreducible pacer (~73us);
  - the [1,S] DVE reciprocal (6.5us/head) serialized the old pipeline;
  - PSUM has exactly 8 banks: lg(2 bufs x [128,1024]) + av(1 x [72,1024])
    + pj(2 x [128,512]) == 8, so projections interleave as PE filler.

Structure:
  - QK projection pair0 up front (DMA-paced), then per pair, per head
    (sequential heads): per sk-tile: logits matmul (K=64) -> ACT exp with
    fused per-partition bias -> attn@V accumulate (K=128, fp8).
    V-projection and next pair's QK projection are emitted into the
    exp-wait slots of the head loop.
  - V_aug has a ones-column per head so av row 64 holds the softmax
    denominator; a per-head DVE row-copy gathers denoms at 32-aligned
    partitions {0,32,64,96} of two den tiles (engine APs require
    32-aligned partition bases).
  - 1/denom: heads 0-3 and 4-5 via batched DVE reciprocals mid-stream;
    heads 6-7 via ACT exp(-ln(x)) in the tail (fast, avoids the banned
    ACT Reciprocal).  Each rc row is staged to a [1,S] partition-0 tile
    (the gpsimd broadcast ucode ignores AP partition bases), then a
    full-128 gpsimd partition_broadcast + DVE muls normalize.
  - Wo runs as a tail phase accumulating the full K=512 contraction in
    PSUM (4 matmuls per [128,512] tile), staged to SBUF alternating
    DVE/ACT copies, output DMA pipelined per tile.

dtypes: all-bf16 matmuls, fp32 PSUM.  (fp8 was measured at 2-4.5%
max-rel error per tensor on this problem -- over the 2e-2 budget -- and
buys no PE throughput without DoubleRow, so it is not used.)
"""

import numpy as np
import ml_dtypes

import concourse.bass as bass
import concourse.tile as tile
from concourse import bacc, mybir
from concourse import bass_utils

F32 = mybir.dt.float32
BF16 = mybir.dt.bfloat16
FP8 = mybir.dt.float8e4
EXP = mybir.ActivationFunctionType.Exp
LN = mybir.ActivationFunctionType.Ln
COPY = mybir.ActivationFunctionType.Copy

S = 1024  # sequence length (tokens)
HID = 1024  # model hidden
DQ = 512  # per-core projected dim (8 heads x 64)
NHL = 8  # local heads per core
DH = 64  # head depth
NK = HID // 128  # 8 contraction tiles over hidden
P = 128
N_CORES = 8
VW = DH + 1  # 65: per-head V_aug block width (64 v + ones col)
BIAS_SHIFT = 0.0  # no shift needed for bf16 e (exp(+-9.5) is fine)

_CACHED_NC = None


def build_program():
    nc = bacc.Bacc("TRN2", target_bir_lowering=False, debug=False)  # v5-ldw
    xt = nc.dram_tensor("xt", [HID, S], BF16, kind="ExternalInput").ap()
    yt = nc.dram_tensor("yt", [HID, S], BF16, kind="ExternalInput").ap()
    # wqp/wkp pair-packed: col = pair*1024 + k*128 + c
    wqp = nc.dram_tensor("wqp", [P, 4 * NK * P], BF16, kind="ExternalInput").ap()
    wkp = nc.dram_tensor("wkp", [P, 4 * NK * P], BF16, kind="ExternalInput").ap()
    wv = nc.dram_tensor("wv", [HID, DQ], BF16, kind="ExternalInput").ap()
    wo = nc.dram_tensor("wo", [DQ, HID], BF16, kind="ExternalInput").ap()
    biasd = nc.dram_tensor("biasd", [P, NK], F32, kind="ExternalInput").ap()
    onesd = nc.dram_tensor("onesd", [P, DH], BF16, kind="ExternalInput").ap()
    out = nc.dram_tensor("out", [S, HID], BF16, kind="ExternalOutput").ap()
    nc.dram_tensor("v5tag", [1, 1], F32, kind="ExternalInput").ap()

    with tile.TileContext(nc) as tc:
        emit_kernel(tc, out, xt, yt, wqp, wkp, wv, wo, biasd, onesd)
    nc.compile()
    return nc


def emit_kernel(tc, out, xt, yt, wqp, wkp, wv, wo, biasd, onesd):
    nc = tc.nc
    with (
        tc.tile_pool(name="wpool", bufs=1) as wpool,
        tc.tile_pool(name="xypool", bufs=1) as xypool,
        tc.tile_pool(name="qkv", bufs=1) as qkvpool,
        tc.tile_pool(name="atp", bufs=1) as atpool,
        tc.tile_pool(name="expp", bufs=3) as exppool,
        tc.tile_pool(name="smallp", bufs=1) as smallpool,
        tc.tile_pool(name="bcp", bufs=2) as bcpool,
    ):
        # ---- input DMA (3 rings; earliest-needed first) ----
        bias_sb = wpool.tile([P, NK], F32, tag="bias")
        nc.gpsimd.dma_start(bias_sb[:], biasd[:])
        vones_sb = wpool.tile([P, DH], BF16, tag="vones")
        nc.gpsimd.dma_start(vones_sb[:], onesd[:])
        wq_sb = []
        wk_sb = []
        for pr in range(4):
            t = wpool.tile([P, NK * P], BF16, tag=f"wq{pr}", name=f"wq{pr}")
            wq_sb.append(t)
            t = wpool.tile([P, NK * P], BF16, tag=f"wk{pr}", name=f"wk{pr}")
            wk_sb.append(t)
        nc.sync.dma_start(wq_sb[0][:], wqp[:, 0 : NK * P])
        nc.scalar.dma_start(wk_sb[0][:], wkp[:, 0 : NK * P])
        # strict priority: (1) pair-0 weights, (2) xt+yt k-interleaved,
        # (3) wv, (4) remaining wq/wk, (5) wo -- 3 rings balanced so the
        # exp stream can start as soon as the DMA window allows (~16us)
        rings = (nc.sync, nc.scalar, nc.gpsimd)
        xt_sb = [None] * NK
        yt_sb = [None] * NK
        for k in range(NK):
            t = xypool.tile([P, S], BF16, tag=f"xt{k}", name=f"xt{k}")
            rings[k % 3].dma_start(t[:], xt[k * P : (k + 1) * P, :])
            xt_sb[k] = t
            t = xypool.tile([P, S], BF16, tag=f"yt{k}", name=f"yt{k}")
            rings[(k + 2) % 3].dma_start(t[:], yt[k * P : (k + 1) * P, :])
            yt_sb[k] = t
        wv_sb = []
        for k in range(NK):
            t = wpool.tile([P, DQ], BF16, tag=f"wv{k}", name=f"wv{k}")
            rings[k % 3].dma_start(t[:], wv[k * P : (k + 1) * P, :])
            wv_sb.append(t)
        for pr in range(1, 4):
            rings[(2 * pr) % 3].dma_start(
                wq_sb[pr][:], wqp[:, pr * NK * P : (pr + 1) * NK * P]
            )
            rings[(2 * pr + 1) % 3].dma_start(
                wk_sb[pr][:], wkp[:, pr * NK * P : (pr + 1) * NK * P]
            )
        wo_sb = []
        for k in range(DQ // P):
            t = wpool.tile([P, HID], BF16, tag=f"wo{k}", name=f"wo{k}")
            rings[k % 3].dma_start(t[:], wo[k * P : (k + 1) * P, :])
            wo_sb.append(t)

        # ---- persistent slabs ----
        qt_sb = [qkvpool.tile([P, S], BF16, tag=f"qt{m}", name=f"qt{m}") for m in range(4)]
        kt_sb = [qkvpool.tile([P, S], BF16, tag=f"kt{m}", name=f"kt{m}") for m in range(4)]
        v_sb = [qkvpool.tile([P, NHL * VW], BF16, tag=f"v{m}", name=f"v{m}") for m in range(8)]
        at_sb = [atpool.tile([P, S], BF16, tag=f"at{m}", name=f"at{m}") for m in range(4)]
        atn_sb = [atpool.tile([P, S], BF16, tag=f"atn{m}", name=f"atn{m}") for m in range(4)]
        # heads 0-3 / 4-7 denoms at rows {0,32,64,96} (32-aligned bases)
        den1_sb = smallpool.tile([P, S], F32, tag="den1")
        den2_sb = smallpool.tile([P, S], F32, tag="den2")
        lden_sb = smallpool.tile([P, S], F32, tag="lden")
        rc1_sb = smallpool.tile([P, S], F32, tag="rc1")
        rc2a_sb = smallpool.tile([P, S], F32, tag="rc2a")
        rc2b_sb = smallpool.tile([P, S], F32, tag="rc2b")
        # per-head staging: broadcast ucode only reads partition 0;
        # bf16 so the broadcast and the normalize mul run at 2x DVE rate
        rcs_sb = [
            smallpool.tile([1, S], BF16, tag=f"rcs{h}", name=f"rcs{h}")
            for h in range(NHL)
        ]

        # ---- PSUM pools: lg 2x[128,1024]=4 banks, av 1x[72,1024]=2,
        # pj 2x[128,512]=2 -> 8 banks exactly.
        pp_lg = tc.alloc_tile_pool(name="pp_lg", bufs=2, space="PSUM")
        pp_av = tc.alloc_tile_pool(name="pp_av", bufs=1, space="PSUM")
        pp_pj = tc.alloc_tile_pool(name="pp_pj", bufs=2, space="PSUM")

        def emit_qkproj_group(pair, which, n):
            """One [128,512] projection psum group: 8 k-matmuls + cast."""
            w_sb = wq_sb[pair] if which == "q" else wk_sb[pair]
            src = xt_sb if which == "q" else yt_sb
            dst = qt_sb[pair] if which == "q" else kt_sb[pair]
            ps = pp_pj.tile([P, 512], F32, tag="pj", name="pj")
            for k in range(NK):
                nc.tensor.matmul(
                    ps[:],
                    w_sb[:, k * P : (k + 1) * P],
                    src[k][:, n * 512 : (n + 1) * 512],
                    start=(k == 0),
                    stop=(k == NK - 1),
                )
            nc.vector.tensor_copy(dst[:, n * 512 : (n + 1) * 512], ps[:])

        def emit_vproj(m):
            """V projection for token-slab m, ones columns appended."""
            ps = pp_pj.tile([P, 512], F32, tag="pj", name="pj")
            for k in range(NK):
                nc.tensor.matmul(
                    ps[:],
                    yt_sb[k][:, m * P : (m + 1) * P],
                    wv_sb[k][:],
                    start=(k == 0),
                    stop=(k == NK - 1),
                )
            dst3 = v_sb[m][:].rearrange("p (h c) -> p h c", c=VW)
            src3 = ps[:].rearrange("p (h c) -> p h c", c=DH)
            nc.vector.tensor_copy(dst3[:, :, 0:DH], src3[:, :, :])
            nc.vector.tensor_copy(
                dst3[:, :, DH:VW],
                vones_sb[:, 0:NHL].rearrange("p (a b) -> p a b", b=1),
            )

        # fillers[(pair, hi, sk)] -> list of thunks emitted after that
        # sk-slot's av matmuls (PE work that overlaps the exp stream).
        fillers = {}
        for sk in range(7):
            fillers.setdefault((0, 0, sk), []).append(
                (lambda m: (lambda: emit_vproj(m)))(sk + 1)
            )
        for g in range(4):
            which, n = ("q", g % 2) if g < 2 else ("k", g % 2)
            fillers.setdefault((0, 1, 2 * g), []).append(
                (lambda w, nn: (lambda: emit_qkproj_group(1, w, nn)))(which, n)
            )
        for hi in range(2):
            for sk in range(NK):
                fillers.setdefault((3, hi, sk), []).append(
                    (lambda: emit_warm(3))
                )
        # heads 0-3: reciprocal + normalize spread over pair-2 head-B
        # slots (head-A's slots already queue QK3 casts on DVE; keeping
        # this work off them lets the av-release cast run promptly)
        fillers.setdefault((2, 1, 0), []).append(
            lambda: nc.vector.reciprocal(rc1_sb[:], den1_sb[:])
        )
        for hh in range(4):
            fillers.setdefault((2, 1, 1 + hh), []).append(
                (lambda h_: (lambda: emit_normalize(h_, rc1_sb, 32 * h_)))(hh)
            )
        # heads 4-5: spread over pair-3 head-A slots
        fillers.setdefault((3, 0, 1), []).append(
            lambda: nc.vector.reciprocal(rc2a_sb[:], den2_sb[:])
        )
        for hh in range(4, 6):
            fillers.setdefault((3, 0, 2 + hh - 4), []).append(
                (lambda h_: (lambda: emit_normalize(h_, rc2a_sb, 32 * (h_ - 4))))(hh)
            )
        # head 6: re-run the reciprocal once its denom (row 64) exists,
        # spread over pair-3 head-B slots
        fillers.setdefault((3, 1, 1), []).append(
            lambda: nc.vector.reciprocal(rc2b_sb[:], den2_sb[:])
        )
        fillers.setdefault((3, 1, 2), []).append(
            lambda: emit_normalize(6, rc2b_sb, 64)
        )
        for pair in (1, 2):
            for g in range(4):
                which, n = ("q", g % 2) if g < 2 else ("k", g % 2)
                hi, sk = divmod(2 * g, 8)
                fillers.setdefault((pair, hi, sk), []).append(
                    (lambda w, nn, pp: (lambda: emit_qkproj_group(pp, w, nn)))(
                        which, n, pair + 1
                    )
                )

        # ---- QK projection pair 0 + V slab 0 (DMA-paced lead-in) ----
        for which in ("q", "k"):
            for n in range(2):
                emit_qkproj_group(0, which, n)
        emit_vproj(0)

        warm_ps = [None]

        def emit_warm(nmm):
            """Junk K=1 matmuls into a scratch pj tile; keeps the PE's HAM
            activity window busy through low-duty stretches so the Wo tail
            runs at 2.4 GHz."""
            if warm_ps[0] is None:
                warm_ps[0] = pp_pj.tile([P, 512], F32, tag="pj", name="warm")
            for _ in range(nmm):
                nc.tensor.matmul(
                    warm_ps[0][0:DH, 0:512],
                    vones_sb[0:1, 0:DH],
                    qt_sb[0][0:1, 0:512],
                    start=True,
                    stop=True,
                )

        def emit_normalize(hh, rc_t, row):
            """Stage rc row to partition 0, broadcast, multiply."""
            pr, hhi = divmod(hh, 2)
            bb = hhi * DH
            nc.gpsimd.dma_start(rcs_sb[hh][:], rc_t[row : row + 1, :])
            bc = bcpool.tile([P, S], BF16, tag="bc", name="bc")
            nc.gpsimd.partition_broadcast(bc[:], rcs_sb[hh][:])
            nc.vector.tensor_mul(
                atn_sb[pr][bb : bb + DH, :],
                at_sb[pr][bb : bb + DH, :],
                bc[bb : bb + DH, :],
            )

        # ---- head loop: sequential heads, ACT-paced exp stream ----
        for pair in range(4):
            for hi in range(2):
                h = 2 * pair + hi
                base = hi * DH
                av = pp_av.tile([VW, S], F32, tag="av", name="av")
                for sk in range(NK):
                    lg = pp_lg.tile([P, S], F32, tag="lg", name="lg")
                    for n in range(2):
                        nc.tensor.matmul(
                            lg[:, n * 512 : (n + 1) * 512],
                            kt_sb[pair][base : base + DH, sk * P : (sk + 1) * P],
                            qt_sb[pair][base : base + DH, n * 512 : (n + 1) * 512],
                            start=True,
                            stop=True,
                        )
                    e = exppool.tile([P, S], BF16, tag="exp", name="exp")
                    nc.scalar.activation(
                        e[:], lg[:], EXP, bias=bias_sb[:, sk : sk + 1]
                    )
                    for n in range(2):
                        nc.tensor.matmul(
                            av[:, n * 512 : (n + 1) * 512],
                            v_sb[sk][:, h * VW : (h + 1) * VW],
                            e[:, n * 512 : (n + 1) * 512],
                            start=(sk == 0),
                            stop=(sk == NK - 1),
                        )
                    for thunk in fillers.get((pair, hi, sk), ()):
                        thunk()
                # gather: unnormalized attn rows -> at slab, denom row -> den
                nc.vector.tensor_copy(
                    at_sb[pair][base : base + DH, :], av[0:DH, :]
                )
                den_t = den1_sb if h < 4 else den2_sb
                row = 32 * (h % 4)
                nc.vector.tensor_copy(
                    den_t[row : row + 1, :], av[DH : DH + 1, :]
                )

        # ---- tail: head 7 via ACT exp(-ln(denom)) on [1,S] rows ----
        nc.scalar.activation(lden_sb[0:1, :], den2_sb[96:97, :], LN)
        nc.scalar.activation(rcs_sb[7][:], lden_sb[0:1, :], EXP, scale=-1.0)
        emit_warm(16)
        bc7 = bcpool.tile([P, S], BF16, tag="bc", name="bc")
        nc.gpsimd.partition_broadcast(bc7[:], rcs_sb[7][:])
        emit_warm(16)
        nc.vector.tensor_mul(
            atn_sb[3][DH:P, :], at_sb[3][DH:P, :], bc7[DH:P, :]
        )

        # ---- Wo: full-contraction PSUM accumulation, staged out ----
        pp_pj.release()
        pp_av.release()
        pp_lg.release()
        pp_wo = tc.alloc_tile_pool(name="pp_wo", bufs=4, space="PSUM")
        with tc.tile_pool(name="ostage", bufs=4) as ostagepool:
            idx = 0
            for m in range(8):
                for n in range(2):
                    ps = pp_wo.tile([P, 512], F32, tag="wops", name="wops")
                    for kp in range(4):
                        nc.tensor.matmul(
                            ps[:],
                            atn_sb[kp][:, m * P : (m + 1) * P],
                            wo_sb[kp][:, n * 512 : (n + 1) * 512],
                            start=(kp == 0),
                            stop=(kp == 3),
                        )
                    st = ostagepool.tile([P, 512], BF16, tag="ost", name="ost")
                    if idx % 2 == 0:
                        nc.vector.tensor_copy(st[:], ps[:])
                    else:
                        nc.scalar.activation(st[:], ps[:], COPY)
                    rings[idx % 3].dma_start(
                        out[m * P : (m + 1) * P, n * 512 : (n + 1) * 512], st[:]
                    )
                    idx += 1
        pp_wo.release()


def _prep_in_maps(x, y, bias, Wq, Wk, Wv, Wo):
    x = np.asarray(x, dtype=np.float32)
    y = np.asarray(y, dtype=np.float32)
    bias = np.asarray(bias, dtype=np.float32)
    Wq = np.asarray(Wq, dtype=np.float32)
    Wk = np.asarray(Wk, dtype=np.float32)
    Wv = np.asarray(Wv, dtype=np.float32)
    Wo = np.asarray(Wo, dtype=np.float32)
    bf = ml_dtypes.bfloat16

    def pack_pairs(W, cols, scale, dt):
        # [HID, 512] -> [128, 4*8*128] with col = pair*1024 + k*128 + c
        Ws = (W[:, cols] * scale).astype(np.float32)
        return np.ascontiguousarray(
            Ws.reshape(NK, P, 4, P).transpose(1, 2, 0, 3).reshape(P, 4 * NK * P)
        ).astype(dt)

    in_maps = []
    for c in range(N_CORES):
        b, hf = divmod(c, 2)
        cols = slice(hf * DQ, (hf + 1) * DQ)
        in_maps.append(
            {
                "xt": np.ascontiguousarray(x[b].T).astype(bf),
                "yt": np.ascontiguousarray(y[b].T).astype(bf),
                "wqp": pack_pairs(Wq, cols, 1.0 / 8.0, bf),
                "wkp": pack_pairs(Wk, cols, 1.0, bf),
                "wv": np.ascontiguousarray(Wv[:, cols]).astype(bf),
                "wo": np.ascontiguousarray(Wo[cols, :]).astype(bf),
                "biasd": np.ascontiguousarray(
                    bias[b, 0, 0].reshape(NK, P).T - BIAS_SHIFT
                ),
                "onesd": np.ones((P, DH), dtype=bf),
                "v5tag": np.zeros((1, 1), dtype=np.float32),
            }
        )
    return in_maps


def get_program():
    global _CACHED_NC
    if _CACHED_NC is None:
        _CACHED_NC = build_program()
    return _CACHED_NC


def kernel(x, y, bias, Wq, Wk, Wv, Wo):
    nc = get_program()
    in_maps = _prep_in_maps(x, y, bias, Wq, Wk, Wv, Wo)
    res = bass_utils.run_bass_kernel_spmd(nc, in_maps, core_ids=list(range(N_CORES)))
    B = 4
    out = np.empty((B, S, HID), dtype=np.float32)
    for b in range(B):
        out[b] = np.asarray(res.results[2 * b]["out"], dtype=np.float32) + np.asarray(
            res.results[2 * b + 1]["out"], dtype=np.float32
        )
    return out
